# revision 1
# baseline (speedup 1.0000x reference)
"""Distributed MHA kernel for one TRN2 chip (8 NeuronCores), Bass/Tile.

Problem: B=4, S=2048, D=1024, H=16 full multi-head attention
(qkv proj -> scaled dot product softmax attention -> o proj).

Sharding (no collectives): core c handles batch b=c//2 and query-token
half c%2 (1024 query tokens).  Each core recomputes K/V projections for
the full 2048 tokens of its batch (+25% PE work, zero cross-core sync).
The host permutes x[b] so the core's query tokens come first; softmax
over keys is permutation invariant, so K/V token order doesn't matter.

On-chip dataflow (per core), all fp32 storage, float32r matmuls:
  x^T [D,S] din-major  -> K^T [dout,tok] head-major   (ACT bias fused)
                       -> V   [tok,dv]   token-major, 65-col head blocks
                          with a ones column (softmax denominator trick)
  per (head, q512): logits^T [k,q] = K_h^T.T @ Q_h^T   (contract hd=64)
                    P^T = exp(0.125 * logits^T)         (ACT, no max sub:
                      logits ~ N(0,1) here, exp is safe in fp32)
                    PV: vals^T[d,q] += V_aug[k,65].T @ P^T[k,q]
                      row 64 of vals^T psum = sum_k P^T = softmax denom
                    normalize by broadcast reciprocal, assemble vals^T
  o proj: out[tok,e] = vals^T[:,tok].T @ o_w^T[:,e]    (DVE bias fused)
"""

import numpy as np

_NC_CACHE = {}


def _build_nc(S, D, H, SQ, use_bf16=True):
    import concourse.bass as bass
    import concourse.mybir as mybir
    import concourse.tile as tile
    from concourse import bacc
    from concourse.bass import ts

    f32 = mybir.dt.float32
    cdt = mybir.dt.bfloat16 if use_bf16 else f32
    Copy = mybir.ActivationFunctionType.Copy
    Exp = mybir.ActivationFunctionType.Exp
    add = mybir.AluOpType.add
    mult = mybir.AluOpType.mult

    P = 128
    hd = D // H            # 64 head dim
    hd1 = hd + 1           # 65: V block + ones column
    ND = D // P            # 8 din/dout chunks
    NT = S // 512          # 4 tok512 chunks (K/V)
    NQ = SQ // 512         # 2 q512 chunks
    NK = S // P            # 16 k-token chunks
    HPC = P // hd          # 2 heads per 128-partition chunk
    NG = D // 512          # 2 dv512 groups
    scale = 1.0 / float(np.sqrt(hd))

    nc = bacc.Bacc(trn_type="TRN2", debug=False)

    xT = nc.declare_dram_parameter("xT", [D, S], cdt, isOutput=False)
    wqT = nc.declare_dram_parameter("wqT", [D, D], cdt, isOutput=False)
    wkT = nc.declare_dram_parameter("wkT", [D, D], cdt, isOutput=False)
    wvT = nc.declare_dram_parameter("wvT", [D, D], cdt, isOutput=False)
    owT = nc.declare_dram_parameter("owT", [D, D], cdt, isOutput=False)
    bq = nc.declare_dram_parameter("bq", [D], f32, isOutput=False)
    bk = nc.declare_dram_parameter("bk", [D], f32, isOutput=False)
    bv = nc.declare_dram_parameter("bv", [D], f32, isOutput=False)
    bo = nc.declare_dram_parameter("bo", [D], f32, isOutput=False)
    out = nc.declare_dram_parameter("out", [SQ, D], f32, isOutput=True)

    # [din, tok] viewed as [p, din_chunk, tok]
    xT_r = xT.ap().rearrange("(c p) s -> p c s", p=P)
    wqT_r = wqT.ap().rearrange("(c p) e -> p c e", p=P)
    wkT_r = wkT.ap().rearrange("(c p) e -> p c e", p=P)
    wvT_r = wvT.ap().rearrange("(c p) e -> p c e", p=P)
    owT_r = owT.ap().rearrange("(c p) e -> p c e", p=P)

    def mm(ps, lhsT, rhs, start, stop):
        nc.tensor.matmul(ps, lhsT, rhs, start=start, stop=stop)

    with tile.TileContext(nc) as tc:
        with (
            tc.tile_pool(name="const", bufs=1) as constp,
            tc.tile_pool(name="kpool", bufs=1) as kpool,
            tc.tile_pool(name="vpool", bufs=1) as vpool,
            tc.tile_pool(name="xpool", bufs=4) as xpool,
            tc.tile_pool(name="wpool", bufs=4) as wpool,
            tc.tile_pool(name="wgpool", bufs=2) as wgpool,
            tc.tile_pool(name="qpool", bufs=2) as qpool,
            tc.tile_pool(name="valspool", bufs=2) as valspool,
            tc.tile_pool(name="ptpool", bufs=4) as ptpool,
            tc.tile_pool(name="opool", bufs=3) as opool,
            tc.tile_pool(name="lpool", bufs=2) as lpool,
            tc.tile_pool(name="lgps", bufs=3, space="PSUM") as lgps,
            tc.tile_pool(name="mmps", bufs=2, space="PSUM") as mmps,
        ):
            # ---- constants: biases ----
            bqs = constp.tile([P, ND], f32)
            nc.sync.dma_start(bqs[:], bq.ap().rearrange("(c p) -> p c", p=P))
            bks = constp.tile([P, ND], f32)
            nc.sync.dma_start(bks[:], bk.ap().rearrange("(c p) -> p c", p=P))
            bvb = constp.tile([P, D], f32)
            nc.sync.dma_start(bvb[:], bv.ap().unsqueeze(0).to_broadcast((P, D)))
            bob = constp.tile([P, D], f32)
            nc.sync.dma_start(bob[:], bo.ap().unsqueeze(0).to_broadcast((P, D)))

            # ---- K^T and V_aug persistent in SBUF (fits in bf16) ----
            ksb = kpool.tile([P, ND, S], cdt)          # K^T [p, dout_chunk, tok]
            vsb = vpool.tile([P, NK, H, hd1], cdt)     # V [tok_p, kchunk, head, 65]
            nc.vector.memset(vsb[:, :, :, hd:hd1], 1.0)  # ones columns

            # ---- x fully resident in bf16, loaded once ----
            xts = []
            for t in range(NT):
                xt = xpool.tile([P, ND, 512], cdt, tag="x")
                nc.sync.dma_start(xt[:], xT_r[:, :, ts(t, 512)])
                xts.append(xt)

            # ---- Q^T for all q512 chunks up front ----
            qsbs = []
            for qi in range(NQ):
                qsb = qpool.tile([P, ND, 512], cdt, tag="q")
                for c in range(ND):
                    wt = wpool.tile([P, ND, P], cdt, tag="w")
                    nc.sync.dma_start(wt[:], wqT_r[:, :, ts(c, P)])
                    ps = mmps.tile([P, 512], f32, tag="mm")
                    for d in range(ND):
                        mm(ps[:], wt[:, d, :], xts[qi][:, d, :],
                           d == 0, d == ND - 1)
                    nc.vector.tensor_scalar_add(qsb[:, c, :], ps[:],
                                                bqs[:, c:c + 1])
                qsbs.append(qsb)

            # ---- V then K per head-group, low head groups first so the
            #      attention for early heads can overlap late projections ----
            for g in range(NG):
                wvg = wgpool.tile([P, ND, 512], cdt, tag="wg")
                nc.sync.dma_start(wvg[:], wvT_r[:, :, ts(g, 512)])
                for t in range(NT):
                    for s in range(4):
                        kc = 4 * t + s
                        ps = mmps.tile([P, 512], f32, tag="mm")
                        for d in range(ND):
                            mm(ps[:], xts[t][:, d, ts(s, P)], wvg[:, d, :],
                               d == 0, d == ND - 1)
                        dst = vsb[:, kc, ts(g, 512 // hd), 0:hd]
                        nc.vector.tensor_tensor(
                            dst,
                            ps[:].rearrange("p (h e) -> p h e", e=hd),
                            bvb[:, ts(g, 512)].rearrange("p (h e) -> p h e", e=hd),
                            op=add)
                # K chunks covering this head group (heads 8g..8g+7)
                for c in range(4 * g, 4 * g + 4):
                    wt = wpool.tile([P, ND, P], cdt, tag="w")
                    nc.sync.dma_start(wt[:], wkT_r[:, :, ts(c, P)])
                    for t in range(NT):
                        ps = mmps.tile([P, 512], f32, tag="mm")
                        for d in range(ND):
                            mm(ps[:], wt[:, d, :], xts[t][:, d, :],
                               d == 0, d == ND - 1)
                        nc.vector.tensor_scalar_add(ksb[:, c, ts(t, 512)],
                                                    ps[:], bks[:, c:c + 1])

            # ---- attention per q512, head-PAIR inner (row-group packed
            #      logits); o-proj(qi) emitted right after its last pair so
            #      it fills PE gaps during qi+1's ACT-paced attention ----
            for qi in range(NQ):
                valsb = valspool.tile([P, ND, 512], cdt, tag="vals")
                for p in range(H // 2):
                    # heads (2p, 2p+1) live at partition offsets (0, 64) of
                    # Q/K chunk p; their K=64 logits matmuls pack into
                    # different PE row groups and run concurrently.
                    pvs = [mmps.tile([hd1, 512], f32, tag="mm",
                                     name=f"pv{p}_{qi}_{j}") for j in range(2)]
                    for kc in range(NK):
                        lg = lgps.tile([P, 2, 512], f32, tag="lg")
                        for j in range(2):
                            off = j * hd
                            mm(lg[:, j, :], ksb[off:off + hd, p, ts(kc, P)],
                               qsbs[qi][off:off + hd, p, :], True, True)
                        pt = ptpool.tile([P, 2, 512], cdt, tag="pt")
                        nc.scalar.activation(pt[:], lg[:], Exp, scale=scale)
                        for j in range(2):
                            mm(pvs[j][:], vsb[:, kc, 2 * p + j, :], pt[:, j, :],
                               kc == 0, kc == NK - 1)
                    for j in range(2):
                        off = j * hd
                        linv = lpool.tile([1, 512], f32, tag="linv")
                        nc.vector.reciprocal(linv[:], pvs[j][hd:hd1, :])
                        lbc = lpool.tile([hd, 512], f32, tag="lbc")
                        nc.gpsimd.partition_broadcast(lbc[:], linv[0:1, :])
                        nc.vector.tensor_tensor(
                            valsb[off:off + hd, p, :], pvs[j][0:hd, :],
                            lbc[:], op=mult)

                # o projection for this q512
                for g in range(NG):
                    owg = wgpool.tile([P, ND, 512], cdt, tag="wg")
                    nc.sync.dma_start(owg[:], owT_r[:, :, ts(g, 512)])
                    for s in range(4):
                        ps = mmps.tile([P, 512], f32, tag="mm")
                        for d in range(ND):
                            mm(ps[:], valsb[:, d, ts(s, P)], owg[:, d, :],
                               d == 0, d == ND - 1)
                        osb = opool.tile([P, 512], f32, tag="o")
                        nc.vector.tensor_tensor(osb[:], ps[:],
                                                bob[:, ts(g, 512)], op=add)
                        nc.sync.dma_start(
                            out.ap()[qi * 512 + s * P: qi * 512 + (s + 1) * P,
                                     ts(g, 512)],
                            osb[:])

    nc.compile()
    return nc


def _get_nc(S, D, H, SQ, use_bf16=True):
    key = (S, D, H, SQ, use_bf16)
    if key not in _NC_CACHE:
        _NC_CACHE[key] = _build_nc(S, D, H, SQ, use_bf16)
    return _NC_CACHE[key]


def _host_prep_weights(qkv_w, qkv_b, o_w, o_b, H, use_bf16=True):
    """Reorder qkv into head-major q/k/v blocks and pre-transpose."""
    import ml_dtypes
    wdt = ml_dtypes.bfloat16 if use_bf16 else np.float32
    D = o_w.shape[0]
    hd = D // H
    qkv3 = qkv_w.reshape(H, 3, hd, D)
    b3 = qkv_b.reshape(H, 3, hd)
    wqT = np.ascontiguousarray(qkv3[:, 0].reshape(D, D).T.astype(wdt))
    wkT = np.ascontiguousarray(qkv3[:, 1].reshape(D, D).T.astype(wdt))
    wvT = np.ascontiguousarray(qkv3[:, 2].reshape(D, D).T.astype(wdt))
    owT = np.ascontiguousarray(o_w.T.astype(wdt))
    return dict(
        wqT=wqT, wkT=wkT, wvT=wvT, owT=owT,
        bq=np.ascontiguousarray(b3[:, 0].reshape(D)),
        bk=np.ascontiguousarray(b3[:, 1].reshape(D)),
        bv=np.ascontiguousarray(b3[:, 2].reshape(D)),
        bo=np.ascontiguousarray(o_b),
    )


def kernel(x, qkv_w, qkv_b, o_w, o_b, _trace=False):
    from concourse.bass_utils import run_bass_kernel_spmd

    x = np.asarray(x, dtype=np.float32)
    qkv_w = np.asarray(qkv_w, dtype=np.float32)
    qkv_b = np.asarray(qkv_b, dtype=np.float32)
    o_w = np.asarray(o_w, dtype=np.float32)
    o_b = np.asarray(o_b, dtype=np.float32)

    B, S, D = x.shape
    H = 16
    n_cores = 8
    halves = n_cores // B           # 2 query-token halves per batch
    SQ = S // halves                # 1024 query tokens per core

    nc = _get_nc(S, D, H, SQ)
    shared = _host_prep_weights(qkv_w, qkv_b, o_w, o_b, H)

    in_maps = []
    for c in range(n_cores):
        b, half = divmod(c, halves)
        # this core's query tokens first; key/value order is irrelevant
        xp = np.concatenate([x[b, half * SQ:(half + 1) * SQ],
                             np.concatenate([x[b, :half * SQ],
                                             x[b, (half + 1) * SQ:]], axis=0)],
                            axis=0)
        m = dict(shared)
        import ml_dtypes
        m["xT"] = np.ascontiguousarray(xp.T.astype(ml_dtypes.bfloat16))
        in_maps.append(m)

    res = run_bass_kernel_spmd(nc, in_maps, list(range(n_cores)),
                               trace=_trace)

    out = np.empty((B, S, D), dtype=np.float32)
    for c in range(n_cores):
        b, half = divmod(c, halves)
        out[b, half * SQ:(half + 1) * SQ] = res.results[c]["out"]
    if _trace:
        return out, res
    return out



# revision 51
# speedup vs baseline: 1.3682x; 1.3682x over previous
"""Distributed MHA kernel for one TRN2 chip (8 NeuronCores), Bass/Tile.

Problem: B=4, S=2048, D=1024, H=16 full multi-head attention
(qkv proj -> scaled dot product softmax attention -> o proj).

Sharding (no collectives): core c handles batch c//2 and head-half c%2
(8 heads).  Each core computes Q/K/V for its 8 heads over the full 2048
tokens, attention, and a PARTIAL o-projection (contracting only its 512
vals dims).  The host sums the two partial outputs per batch and adds
o_b during unshard.

Per-core dataflow (bf16 matmuls, fp32 psum):
  xT [D,S] -> K^T,Q^T [dout,tok] head-pair-major, V [tok,dv] with a
  ones column per head (softmax denominator rides the PV matmul).
  per (head, q512):  logits^T [k,q] = K_h^T.T @ Q_h^T   (contract 64)
    P^T = exp(0.125 * logits^T)      (ACT; no max-sub: logits safe)
    PV:  vals[q128, 65] += P^T[k, q128].T @ V_aug[k, 65]
         (full 128-wide contract AND output: 65 charged rows/k-chunk)
    normalize by column 64, DMA-xbar transpose to valsT [d, q]
  o proj partial: out[tok, e] = valsT.T @ owT   (no bias; host adds)
Loop order is head-outer / q-chunk-inner so K/V/Q projections spread
across the whole timeline as PE filler (exp on ACT is the pacer), and
o-proj(qi) fills the last head's windows.  Fillers interleave at
~0.5-2us granularity to keep the PE p-state ramped.
"""

import numpy as np

_NC_CACHE = {}


def _build_nc(S, D, HC, use_bf16=True):
    import concourse.bass as bass
    import concourse.mybir as mybir
    import concourse.tile as tile
    from concourse import bacc
    from concourse.bass import ts

    f32 = mybir.dt.float32
    cdt = mybir.dt.bfloat16 if use_bf16 else f32
    Exp = mybir.ActivationFunctionType.Exp
    add = mybir.AluOpType.add

    P = 128
    hd = 64                 # head dim
    hd1 = hd + 1            # V block + ones column
    ND = D // P             # 8 din chunks
    DH = HC * hd            # 512 dout per core
    NC_ = DH // P           # 4 dout chunks (head pairs)
    NT = S // 512           # 4 tok512 chunks
    NK = S // P             # 16 k-token chunks
    NQ = S // 512           # 4 q512 chunks (full S on every core)
    scale = 1.0 / float(np.sqrt(hd))

    nc = bacc.Bacc(trn_type="TRN2", debug=False)

    xT = nc.declare_dram_parameter("xT", [D, S], cdt, isOutput=False)
    wqT = nc.declare_dram_parameter("wqT", [D, DH], cdt, isOutput=False)
    wkT = nc.declare_dram_parameter("wkT", [D, DH], cdt, isOutput=False)
    wvT = nc.declare_dram_parameter("wvT", [D, DH], cdt, isOutput=False)
    owT = nc.declare_dram_parameter("owT", [DH, D], cdt, isOutput=False)
    bq = nc.declare_dram_parameter("bq", [DH], f32, isOutput=False)
    bk = nc.declare_dram_parameter("bk", [DH], f32, isOutput=False)
    bv = nc.declare_dram_parameter("bv", [DH], f32, isOutput=False)
    out = nc.declare_dram_parameter("out", [S, D], f32, isOutput=True)

    xT_r = xT.ap().rearrange("(c p) s -> p c s", p=P)
    wqT_r = wqT.ap().rearrange("(c p) e -> p c e", p=P)
    wkT_r = wkT.ap().rearrange("(c p) e -> p c e", p=P)
    wvT_r = wvT.ap().rearrange("(c p) e -> p c e", p=P)
    owT_r = owT.ap().rearrange("(c p) e -> p c e", p=P)

    def mm(ps, lhsT, rhs, start, stop):
        nc.tensor.matmul(ps, lhsT, rhs, start=start, stop=stop)

    with tile.TileContext(nc) as tc:
        with (
            tc.tile_pool(name="const", bufs=1) as constp,
            tc.tile_pool(name="wpool", bufs=1) as wpool,
            tc.tile_pool(name="xpool", bufs=4) as xpool,
            tc.tile_pool(name="qkpool", bufs=1) as qkpool,
            tc.tile_pool(name="vpool", bufs=1) as vpool,
            tc.tile_pool(name="vtpool", bufs=1) as vtpool,
            tc.tile_pool(name="valspool", bufs=6) as valspool,
            tc.tile_pool(name="ptpool", bufs=3) as ptpool,
            tc.tile_pool(name="linpool", bufs=2) as linpool,
            tc.tile_pool(name="opool", bufs=3) as opool,
            tc.tile_pool(name="mmps", bufs=2, space="PSUM") as mmps,
            tc.tile_pool(name="lgps", bufs=2, space="PSUM") as lgps,
            tc.tile_pool(name="pvps", bufs=2, space="PSUM") as pvps,
        ):
            # ---- weights / x: chunk-0 slices first for a fast start ----
            wks = wpool.tile([P, ND, DH], cdt, tag="wk")
            nc.sync.dma_start(wks[:, :, 0:P], wkT_r[:, :, 0:P])
            xts = []
            for t in range(NT):
                xt = xpool.tile([P, ND, 512], cdt, tag="x", name=f"x{t}")
                xts.append(xt)
            nc.sync.dma_start(xts[0][:], xT_r[:, :, 0:512])
            wqs = wpool.tile([P, ND, DH], cdt, tag="wq")
            nc.sync.dma_start(wqs[:, :, 0:P], wqT_r[:, :, 0:P])
            bqs = constp.tile([P, NC_], f32)
            nc.sync.dma_start(bqs[:], bq.ap().rearrange("(c p) -> p c", p=P))
            bks = constp.tile([P, NC_], f32)
            nc.sync.dma_start(bks[:], bk.ap().rearrange("(c p) -> p c", p=P))
            nc.sync.dma_start(xts[1][:], xT_r[:, :, ts(1, 512)])
            bvb = constp.tile([P, DH], f32)
            nc.sync.dma_start(bvb[:], bv.ap().unsqueeze(0).to_broadcast((P, DH)))
            wvs = wpool.tile([P, ND, DH], cdt, tag="wv")
            nc.sync.dma_start(wvs[:, :, 0:P], wvT_r[:, :, 0:P])
            nc.sync.dma_start(xts[2][:], xT_r[:, :, ts(2, 512)])
            nc.sync.dma_start(xts[3][:], xT_r[:, :, ts(3, 512)])
            nc.sync.dma_start(wks[:, :, P:DH], wkT_r[:, :, P:DH])
            nc.sync.dma_start(wqs[:, :, P:DH], wqT_r[:, :, P:DH])
            nc.sync.dma_start(wvs[:, :, P:DH], wvT_r[:, :, P:DH])
            ows = wpool.tile([P, NC_, D], cdt, tag="ow")
            nc.sync.dma_start(ows[:], owT_r[:])

            # ---- persistent SBUF state ----
            qsb = qkpool.tile([P, NC_, S], cdt, tag="q")
            ksb = qkpool.tile([P, NC_, S], cdt, tag="k")
            vsb = vpool.tile([P, NK, HC, hd1], cdt)
            nc.vector.memset(vsb[:, :, :, hd:hd1], 1.0)
            valsT = vtpool.tile([P, NC_, S], cdt)

            # ---- filler units: PE proj work interleaved between attention
            # matmuls.  Each unit is split into ~430ns sub-steps queued with
            # (earliest, deadline) slot keys; a sub-step is force-emitted
            # before its first consumer (correctness: the per-engine streams
            # are in-order, so a consumer emitted before its producer would
            # deadlock), and pulled early on a ~400ns/slot credit budget to
            # keep the PE p-state ramped while exp paces ACT.
            from collections import deque

            def kq_unit(c, t, w, b, dst):
                def go():
                    ps = mmps.tile([P, 512], f32, tag="mm",
                                   name=f"p{w is wqs}_{c}_{t}")
                    for d in range(ND):
                        mm(ps[:], w[:, d, ts(c, P)], xts[t][:, d, :],
                           d == 0, d == ND - 1)
                    nc.vector.tensor_scalar_add(
                        dst[:, c, ts(t, 512)], ps[:], b[:, c:c + 1])
                return go

            def v_step(kc, p):
                def go():
                    t, s = kc // 4, kc % 4
                    ps = mmps.tile([P, 512], f32, tag="mm", name=f"vp{kc}_{p}")
                    for d in range(ND):
                        mm(ps[:, 0:P], xts[t][:, d, ts(s, P)],
                           wvs[:, d, ts(p, P)], d == 0, d == ND - 1)
                    nc.vector.tensor_tensor(
                        vsb[:, kc, 2 * p:2 * p + 2, 0:hd],
                        ps[:, 0:P].rearrange("p (h e) -> p h e", e=hd),
                        bvb[:, ts(p, P)].rearrange("p (h e) -> p h e", e=hd),
                        op=add)
                return go

            # o-projection in two stages: stage1 contracts head-pairs 0-2
            # (can run as soon as those pairs' valsT(qi) are transposed,
            # well before pair 3's attention), stage2 adds the dc=3 term
            # (one 213ns matmul) and stores.  Keeps pair-3's windows light.
            obuf = {}

            def o_stage1(qi, tc, eg):
                def go():
                    ps = mmps.tile([P, 512], f32, tag="mm",
                                   name=f"op{qi}_{tc}_{eg}")
                    for dc in range(NC_ - 1):
                        mm(ps[:], valsT[:, dc, qi * 512 + tc * P:
                                        qi * 512 + (tc + 1) * P],
                           ows[:, dc, ts(eg, 512)], dc == 0, dc == NC_ - 2)
                    ob = opool.tile([P, 512], cdt, tag="ob",
                                    name=f"ob{qi}_{tc}_{eg}", bufs=32)
                    obuf[(qi, tc, eg)] = ob
                    nc.vector.tensor_copy(ob[:], ps[:])
                return go

            def o_stage2(qi, tc, eg):
                def go():
                    ps = mmps.tile([P, 512], f32, tag="mm",
                                   name=f"oq{qi}_{tc}_{eg}")
                    mm(ps[:], valsT[:, NC_ - 1, qi * 512 + tc * P:
                                    qi * 512 + (tc + 1) * P],
                       ows[:, NC_ - 1, ts(eg, 512)], True, True)
                    osb = opool.tile([P, 512], f32, tag="o",
                                     name=f"os{qi}_{tc}_{eg}")
                    nc.vector.tensor_tensor(osb[:], ps[:],
                                            obuf[(qi, tc, eg)][:], op=add)
                    nc.sync.dma_start(
                        out.ap()[qi * 512 + tc * P: qi * 512 + (tc + 1) * P,
                                 ts(eg, 512)],
                        osb[:])
                return go

            # build queue entries: (earliest, deadline, fn).  Slot keys are
            # (window, kcp, phase); window = (c*NQ + qi)*2 + parity; phase 0
            # = before that slot's logits, phase 1 = after its exp (so
            # forced V/o units never delay the logits feeding ACT).
            # Deadlines sit one window before first use where possible.
            entries = []
            WPC = NQ * 2               # windows per pair
            NKP = NK // 2              # 8 kc-pairs
            for c in range(NC_):
                early = (max(c - 1, 0) * WPC, 0, 0)
                w0 = c * WPC
                for t in range(NT):
                    if (c, t) == (0, 0):
                        continue       # prologue
                    dl = (w0 - 1, 2 * t, 0) if c else (0, 2 * t, 0)
                    entries.append((early, dl,
                                    kq_unit(c, t, wks, bks, ksb)))
                for t in range(NT):
                    if (c, t) == (0, 0):
                        continue
                    dl = (max(w0 + 2 * t - 1, 0), 4, 0)
                    entries.append((early, dl,
                                    kq_unit(c, t, wqs, bqs, qsb)))
                for kc in range(NK):
                    dl = (max(w0 - 1, 0), min(kc // 2 + 1, NKP - 1), 1)
                    entries.append((early, dl, v_step(kc, c)))
            W3 = (NC_ - 1) * WPC       # first window of pair 3
            for qi in range(NQ):
                # stage1 needs valsT(qi) for pairs 0..2: ready after window
                # (NC_-2)*WPC + qi*2 + 1
                e1 = ((NC_ - 2) * WPC + qi * 2 + 2, 0, 0)
                for tc in range(4):
                    for eg in range(2):
                        k = tc * 2 + eg
                        entries.append((e1, (W3 + qi, 1 + (k % 4) * 2, 1),
                                        o_stage1(qi, tc, eg)))
            for qi in range(NQ - 1):   # stage2(qi) forced into pair-3 qi+1
                for tc in range(4):
                    for eg in range(2):
                        k = tc * 2 + eg
                        key = (W3 + (qi + 1) * 2 + k // 4,
                               1 + (k % 4) * 2, 1)
                        entries.append((key, key, o_stage2(qi, tc, eg)))
            entries.sort(key=lambda e: (e[1], e[0]))
            queue = deque(entries)

            def drain(cur):
                while queue and queue[0][1] <= cur:
                    queue.popleft()[2]()

            def pull(cur):
                # at most one unit per slot, up to six windows ahead
                if queue:
                    early, dl, fn = queue[0]
                    if early <= cur and dl <= (cur[0] + 6, 99, 9):
                        queue.popleft()
                        fn()

            # prologue: minimum to start (h0, qi0) attention
            kq_unit(0, 0, wks, bks, ksb)()
            kq_unit(0, 0, wqs, bqs, qsb)()

            # ---- attention: head-pair outer, q512 mid, parity inner ----
            vals_pair = {}             # (c, qi) -> [P, qc, 2*hd] tile
            for c in range(NC_):
              for qi in range(NQ):
                for par in range(2):
                    h = 2 * c + par
                    off = par * hd
                    w = (c * NQ + qi) * 2 + par
                    pv = pvps.tile([P, NQ, hd1], f32, tag="pv",
                                   padded_shape=[P, NQ, P],
                                   name=f"pv{h}_{qi}")
                    pts = [None] * NKP
                    for kcp in range(NKP):
                        drain((w, kcp, 0))
                        lg = lgps.tile([P, 2, 512], f32, tag="lg",
                                       name=f"lg{h}_{qi}_{kcp}")
                        for j in range(2):
                            kc = 2 * kcp + j
                            mm(lg[:, j, :], ksb[off:off + hd, c, ts(kc, P)],
                               qsb[off:off + hd, c, ts(qi, 512)], True, True)
                        pt = ptpool.tile([P, 2, 512], cdt, tag="pt",
                                         name=f"pt{h}_{qi}_{kcp}")
                        nc.scalar.activation(pt[:], lg[:], Exp, scale=scale)
                        pts[kcp] = pt
                        drain((w, kcp, 1))
                        pull((w, kcp, 1))
                        if kcp > 0:
                            pj = pts[kcp - 1]
                            for j in range(2):
                                kc = 2 * (kcp - 1) + j
                                for qc in range(4):
                                    mm(pv[:, qc, 0:hd1],
                                       pj[:, j, ts(qc, P)],
                                       vsb[:, kc, h, :],
                                       kcp == 1 and j == 0 and qc == 0,
                                       False)
                    drain((w, NKP, 0))
                    # normalize by the ones-column sum into the pair tile;
                    # after the odd head, xbar-transpose the full 128-wide
                    # pair tile (transpose needs 128x128 xbar tiles).  The
                    # very last window pipelines per q-chunk straight into
                    # its own o-projection to shorten the drain tail.
                    last = (c == NC_ - 1 and qi == NQ - 1 and par == 1)
                    linv = linpool.tile([P, NQ], f32, tag="lin",
                                        name=f"linv{h}_{qi}")
                    if par == 0:
                        vals_pair[(c, qi)] = valspool.tile(
                            [P, NQ, 2 * hd], cdt, tag="vals",
                            name=f"vals{c}_{qi}")
                    vals = vals_pair[(c, qi)]
                    for j in range(2):
                        kc = 2 * (NKP - 1) + j
                        for qc in range(NQ):
                            mm(pv[:, qc, 0:hd1],
                               pts[NKP - 1][:, j, ts(qc, P)],
                               vsb[:, kc, h, :],
                               False, j == 1 and qc == NQ - 1)
                    for qc in range(NQ):
                        nc.vector.reciprocal(linv[:, qc:qc + 1],
                                             pv[:, qc, hd:hd1])
                        nc.vector.tensor_scalar_mul(vals[:, qc, off:off + hd],
                                                    pv[:, qc, 0:hd],
                                                    linv[:, qc:qc + 1])
                        if par == 1:
                            nc.sync.dma_start_transpose(
                                valsT[:, c, qi * 512 + qc * P:
                                      qi * 512 + (qc + 1) * P],
                                vals[:, qc, :])
                        if last and qc > 0:
                            o_stage2(qi, qc - 1, 0)()
                            o_stage2(qi, qc - 1, 1)()
                    if last:
                        o_stage2(qi, NQ - 1, 0)()
                        o_stage2(qi, NQ - 1, 1)()
            drain((NC_ * WPC, 0, 0))

    nc.compile()
    return nc


def _get_nc(S, D, H=16, SQ=None, use_bf16=True):
    key = (S, D, H, use_bf16)
    if key not in _NC_CACHE:
        _NC_CACHE[key] = _build_nc(S, D, H // 2, use_bf16)
    return _NC_CACHE[key]


def _host_prep(x, qkv_w, qkv_b, o_w, H, use_bf16=True):
    """Per-core weight slices: head-half hh gets heads 8hh..8hh+7."""
    import ml_dtypes
    wdt = ml_dtypes.bfloat16 if use_bf16 else np.float32
    D = o_w.shape[0]
    hd = D // H
    HC = H // 2
    DH = HC * hd
    qkv3 = qkv_w.reshape(H, 3, hd, D)
    b3 = qkv_b.reshape(H, 3, hd)
    shared = []
    for hh in range(2):
        hs = slice(hh * HC, (hh + 1) * HC)
        ds = slice(hh * DH, (hh + 1) * DH)
        shared.append(dict(
            wqT=np.ascontiguousarray(qkv3[hs, 0].reshape(DH, D).T.astype(wdt)),
            wkT=np.ascontiguousarray(qkv3[hs, 1].reshape(DH, D).T.astype(wdt)),
            wvT=np.ascontiguousarray(qkv3[hs, 2].reshape(DH, D).T.astype(wdt)),
            owT=np.ascontiguousarray(o_w.T[ds, :].astype(wdt)),
            bq=np.ascontiguousarray(b3[hs, 0].reshape(DH)),
            bk=np.ascontiguousarray(b3[hs, 1].reshape(DH)),
            bv=np.ascontiguousarray(b3[hs, 2].reshape(DH)),
        ))
    xTs = [np.ascontiguousarray(x[b].T.astype(wdt)) for b in range(x.shape[0])]
    return shared, xTs


def kernel(x, qkv_w, qkv_b, o_w, o_b, _trace=False):
    from concourse.bass_utils import run_bass_kernel_spmd

    x = np.asarray(x, dtype=np.float32)
    qkv_w = np.asarray(qkv_w, dtype=np.float32)
    qkv_b = np.asarray(qkv_b, dtype=np.float32)
    o_w = np.asarray(o_w, dtype=np.float32)
    o_b = np.asarray(o_b, dtype=np.float32)

    B, S, D = x.shape
    H = 16
    n_cores = 8

    nc = _get_nc(S, D, H)
    shared, xTs = _host_prep(x, qkv_w, qkv_b, o_w, H)

    in_maps = []
    for c in range(n_cores):
        b, hh = divmod(c, 2)
        m = dict(shared[hh])
        m["xT"] = xTs[b]
        in_maps.append(m)

    res = run_bass_kernel_spmd(nc, in_maps, list(range(n_cores)),
                               trace=_trace)

    out = np.empty((B, S, D), dtype=np.float32)
    for b in range(B):
        np.add(res.results[2 * b]["out"], res.results[2 * b + 1]["out"],
               out=out[b])
        out[b] += o_b
    if _trace:
        return out, res
    return out


# revision 63
# speedup vs baseline: 1.4321x; 1.0467x over previous
"""Distributed MHA kernel for one TRN2 chip (8 NeuronCores), Bass/Tile.

Problem: B=4, S=2048, D=1024, H=16 full multi-head attention
(qkv proj -> scaled dot product softmax attention -> o proj).

Sharding (no collectives): core c handles batch c//2 and head-half c%2
(8 heads).  Each core computes Q/K/V for its 8 heads over the full 2048
tokens, attention, and a PARTIAL o-projection (contracting only its 512
vals dims).  The host sums the two partial outputs per batch and adds
o_b during unshard.

Per-core dataflow (bf16 matmuls, fp32 psum):
  xT [D,S] -> K^T,Q^T [dout,tok] head-pair-major, V [tok,dv] with a
  ones column per head (softmax denominator rides the PV matmul).
  per (head, q512):  logits^T [k,q] = K_h^T.T @ Q_h^T   (contract 64)
    P^T = exp(0.125 * logits^T)      (ACT; no max-sub: logits safe)
    PV:  vals[q128, 65] += P^T[k, q128].T @ V_aug[k, 65]
         (full 128-wide contract AND output: 65 charged rows/k-chunk)
    normalize by column 64, DMA-xbar transpose to valsT [d, q]
  o proj partial: out[tok, e] = valsT.T @ owT   (no bias; host adds)
Loop order is head-outer / q-chunk-inner so K/V/Q projections spread
across the whole timeline as PE filler (exp on ACT is the pacer), and
o-proj(qi) fills the last head's windows.  Fillers interleave at
~0.5-2us granularity to keep the PE p-state ramped.
"""

import numpy as np

_NC_CACHE = {}


def _build_nc(S, D, HC, use_bf16=True):
    import concourse.bass as bass
    import concourse.mybir as mybir
    import concourse.tile as tile
    from concourse import bacc
    from concourse.bass import ts

    f32 = mybir.dt.float32
    cdt = mybir.dt.bfloat16 if use_bf16 else f32
    Exp = mybir.ActivationFunctionType.Exp
    add = mybir.AluOpType.add

    P = 128
    hd = 64                 # head dim
    hd1 = hd + 1            # V block + ones column
    ND = D // P             # 8 din chunks
    DH = HC * hd            # 512 dout per core
    NC_ = DH // P           # 4 dout chunks (head pairs)
    NT = S // 512           # 4 tok512 chunks
    NK = S // P             # 16 k-token chunks
    NQ = S // 512           # 4 q512 chunks (full S on every core)
    scale = 1.0 / float(np.sqrt(hd))

    nc = bacc.Bacc(trn_type="TRN2", debug=False)

    xT = nc.declare_dram_parameter("xT", [D, S], cdt, isOutput=False)
    wqT = nc.declare_dram_parameter("wqT", [D, DH], cdt, isOutput=False)
    wkT = nc.declare_dram_parameter("wkT", [D, DH], cdt, isOutput=False)
    wvT = nc.declare_dram_parameter("wvT", [D, DH], cdt, isOutput=False)
    owT = nc.declare_dram_parameter("owT", [DH, D], cdt, isOutput=False)
    bq = nc.declare_dram_parameter("bq", [DH], f32, isOutput=False)
    bk = nc.declare_dram_parameter("bk", [DH], f32, isOutput=False)
    bv = nc.declare_dram_parameter("bv", [DH], f32, isOutput=False)
    out = nc.declare_dram_parameter("out", [S, D], f32, isOutput=True)

    xT_r = xT.ap().rearrange("(c p) s -> p c s", p=P)
    wqT_r = wqT.ap().rearrange("(c p) e -> p c e", p=P)
    wkT_r = wkT.ap().rearrange("(c p) e -> p c e", p=P)
    wvT_r = wvT.ap().rearrange("(c p) e -> p c e", p=P)
    owT_r = owT.ap().rearrange("(c p) e -> p c e", p=P)

    def mm(ps, lhsT, rhs, start, stop):
        nc.tensor.matmul(ps, lhsT, rhs, start=start, stop=stop)

    with tile.TileContext(nc) as tc:
        with (
            tc.tile_pool(name="const", bufs=1) as constp,
            tc.tile_pool(name="wpool", bufs=1) as wpool,
            tc.tile_pool(name="xpool", bufs=4) as xpool,
            tc.tile_pool(name="qkpool", bufs=1) as qkpool,
            tc.tile_pool(name="vpool", bufs=1) as vpool,
            tc.tile_pool(name="vtpool", bufs=1) as vtpool,
            tc.tile_pool(name="valspool", bufs=8) as valspool,
            tc.tile_pool(name="ptpool", bufs=4) as ptpool,
            tc.tile_pool(name="linpool", bufs=2) as linpool,
            tc.tile_pool(name="opool", bufs=3) as opool,
            tc.tile_pool(name="mmps", bufs=2, space="PSUM") as mmps,
            tc.tile_pool(name="lgps", bufs=2, space="PSUM") as lgps,
            tc.tile_pool(name="pvps", bufs=2, space="PSUM") as pvps,
        ):
            # ---- weights / x: chunk-0 slices first for a fast start ----
            wks = wpool.tile([P, ND, DH], cdt, tag="wk")
            nc.sync.dma_start(wks[:, :, 0:P], wkT_r[:, :, 0:P])
            xts = []
            for t in range(NT):
                xt = xpool.tile([P, ND, 512], cdt, tag="x", name=f"x{t}")
                xts.append(xt)
            nc.sync.dma_start(xts[0][:], xT_r[:, :, 0:512])
            wqs = wpool.tile([P, ND, DH], cdt, tag="wq")
            nc.sync.dma_start(wqs[:, :, 0:P], wqT_r[:, :, 0:P])
            bqs = constp.tile([P, NC_], f32)
            nc.sync.dma_start(bqs[:], bq.ap().rearrange("(c p) -> p c", p=P))
            bks = constp.tile([P, NC_], f32)
            nc.sync.dma_start(bks[:], bk.ap().rearrange("(c p) -> p c", p=P))
            nc.sync.dma_start(xts[1][:], xT_r[:, :, ts(1, 512)])
            bvb = constp.tile([P, DH], f32)
            nc.sync.dma_start(bvb[:], bv.ap().unsqueeze(0).to_broadcast((P, DH)))
            wvs = wpool.tile([P, ND, DH], cdt, tag="wv")
            nc.sync.dma_start(wvs[:, :, 0:P], wvT_r[:, :, 0:P])
            nc.sync.dma_start(xts[2][:], xT_r[:, :, ts(2, 512)])
            nc.sync.dma_start(xts[3][:], xT_r[:, :, ts(3, 512)])
            nc.sync.dma_start(wks[:, :, P:DH], wkT_r[:, :, P:DH])
            nc.sync.dma_start(wqs[:, :, P:DH], wqT_r[:, :, P:DH])
            nc.sync.dma_start(wvs[:, :, P:DH], wvT_r[:, :, P:DH])
            ows = wpool.tile([P, NC_, D], cdt, tag="ow")
            nc.sync.dma_start(ows[:], owT_r[:])

            # ---- persistent SBUF state ----
            qsb = qkpool.tile([P, NC_, S], cdt, tag="q")
            ksb = qkpool.tile([P, NC_, S], cdt, tag="k")
            vsb = vpool.tile([P, NK, HC, hd1], cdt)
            nc.vector.memset(vsb[:, :, :, hd:hd1], 1.0)
            valsT = vtpool.tile([P, NC_, S], cdt)

            # ---- filler units: PE proj work interleaved between attention
            # matmuls.  Each unit is split into ~430ns sub-steps queued with
            # (earliest, deadline) slot keys; a sub-step is force-emitted
            # before its first consumer (correctness: the per-engine streams
            # are in-order, so a consumer emitted before its producer would
            # deadlock), and pulled early on a ~400ns/slot credit budget to
            # keep the PE p-state ramped while exp paces ACT.
            from collections import deque

            def kq_unit(c, t, w, b, dst):
                def go():
                    ps = mmps.tile([P, 512], f32, tag="mm",
                                   name=f"p{w is wqs}_{c}_{t}")
                    for d in range(ND):
                        mm(ps[:], w[:, d, ts(c, P)], xts[t][:, d, :],
                           d == 0, d == ND - 1)
                    nc.vector.tensor_scalar_add(
                        dst[:, c, ts(t, 512)], ps[:], b[:, c:c + 1])
                return go

            def v_step(kc, p):
                def go():
                    t, s = kc // 4, kc % 4
                    ps = mmps.tile([P, 512], f32, tag="mm", name=f"vp{kc}_{p}")
                    for d in range(ND):
                        mm(ps[:, 0:P], xts[t][:, d, ts(s, P)],
                           wvs[:, d, ts(p, P)], d == 0, d == ND - 1)
                    nc.vector.tensor_tensor(
                        vsb[:, kc, 2 * p:2 * p + 2, 0:hd],
                        ps[:, 0:P].rearrange("p (h e) -> p h e", e=hd),
                        bvb[:, ts(p, P)].rearrange("p (h e) -> p h e", e=hd),
                        op=add)
                return go

            # o-projection in two stages: stage1 contracts head-pairs 0-2
            # (can run as soon as those pairs' valsT(qi) are transposed,
            # well before pair 3's attention), stage2 adds the dc=3 term
            # (one 213ns matmul) and stores.  Keeps pair-3's windows light.
            obuf = {}

            def o_stage1(qi, tc, eg):
                def go():
                    ps = mmps.tile([P, 512], f32, tag="mm",
                                   name=f"op{qi}_{tc}_{eg}")
                    for dc in range(NC_ - 1):
                        mm(ps[:], valsT[:, dc, qi * 512 + tc * P:
                                        qi * 512 + (tc + 1) * P],
                           ows[:, dc, ts(eg, 512)], dc == 0, dc == NC_ - 2)
                    ob = opool.tile([P, 512], cdt, tag="ob",
                                    name=f"ob{qi}_{tc}_{eg}", bufs=32)
                    obuf[(qi, tc, eg)] = ob
                    nc.vector.tensor_copy(ob[:], ps[:])
                return go

            def o_stage2(qi, tc, eg):
                def go():
                    ps = mmps.tile([P, 512], f32, tag="mm",
                                   name=f"oq{qi}_{tc}_{eg}")
                    mm(ps[:], valsT[:, NC_ - 1, qi * 512 + tc * P:
                                    qi * 512 + (tc + 1) * P],
                       ows[:, NC_ - 1, ts(eg, 512)], True, True)
                    osb = opool.tile([P, 512], f32, tag="o",
                                     name=f"os{qi}_{tc}_{eg}")
                    nc.vector.tensor_tensor(osb[:], ps[:],
                                            obuf[(qi, tc, eg)][:], op=add)
                    nc.sync.dma_start(
                        out.ap()[qi * 512 + tc * P: qi * 512 + (tc + 1) * P,
                                 ts(eg, 512)],
                        osb[:])
                return go

            # build queue entries: (earliest, deadline, fn).  Slot keys are
            # (window, kcp, phase); window = (c*NQ + qi)*2 + parity; phase 0
            # = before that slot's logits, phase 1 = after its exp (so
            # forced V/o units never delay the logits feeding ACT).
            # Deadlines sit one window before first use where possible.
            entries = []
            WPC = NQ * 2               # windows per pair
            NKP = NK // 2              # 8 kc-pairs
            for c in range(NC_):
                early = (max(c - 1, 0) * WPC, 0, 0)
                w0 = c * WPC
                for t in range(NT):
                    if (c, t) == (0, 0):
                        continue       # prologue
                    dl = (w0 - 1, 2 * t, 0) if c else (0, 2 * t, 0)
                    entries.append((early, dl,
                                    kq_unit(c, t, wks, bks, ksb)))
                for t in range(NT):
                    if (c, t) == (0, 0):
                        continue
                    dl = (max(w0 + 2 * t - 1, 0), 4, 0)
                    entries.append((early, dl,
                                    kq_unit(c, t, wqs, bqs, qsb)))
                for kc in range(NK):
                    dl = (max(w0 - 1, 0), min(kc // 2 + 1, NKP - 1), 1)
                    entries.append((early, dl, v_step(kc, c)))
            W3 = (NC_ - 1) * WPC       # first window of pair 3
            for qi in range(NQ):
                # stage1 needs valsT(qi) for pairs 0..2: ready after window
                # (NC_-2)*WPC + qi*2 + 1
                e1 = ((NC_ - 2) * WPC + qi * 2 + 2, 0, 0)
                for tc in range(4):
                    for eg in range(2):
                        k = tc * 2 + eg
                        entries.append((e1, (W3 + qi, 1 + (k % 4) * 2, 1),
                                        o_stage1(qi, tc, eg)))
            for qi in range(NQ - 1):   # stage2(qi) forced into pair-3 qi+1
                for tc in range(4):
                    for eg in range(2):
                        k = tc * 2 + eg
                        key = (W3 + (qi + 1) * 2 + k // 4,
                               1 + (k % 4) * 2, 1)
                        entries.append((key, key, o_stage2(qi, tc, eg)))
            entries.sort(key=lambda e: (e[1], e[0]))
            queue = deque(entries)

            def drain(cur):
                while queue and queue[0][1] <= cur:
                    queue.popleft()[2]()

            def pull(cur):
                # at most one unit per slot, due this window
                if queue:
                    early, dl, fn = queue[0]
                    if early <= cur and dl <= (cur[0], 99, 9):
                        queue.popleft()
                        fn()

            # prologue: minimum to start (h0, qi0) attention
            kq_unit(0, 0, wks, bks, ksb)()
            kq_unit(0, 0, wqs, bqs, qsb)()

            # ---- attention: head-pair outer, q512 mid, parity inner ----
            vals_pair = {}             # (c, qi) -> [P, qc, 2*hd] tile
            for c in range(NC_):
              for qi in range(NQ):
                for par in range(2):
                    h = 2 * c + par
                    off = par * hd
                    w = (c * NQ + qi) * 2 + par
                    pv = pvps.tile([P, NQ, hd1], f32, tag="pv",
                                   padded_shape=[P, NQ, P],
                                   name=f"pv{h}_{qi}")
                    pts = [None] * NKP
                    for kcp in range(NKP):
                        drain((w, kcp, 0))
                        lg = lgps.tile([P, 2, 512], f32, tag="lg",
                                       name=f"lg{h}_{qi}_{kcp}")
                        for j in range(2):
                            kc = 2 * kcp + j
                            mm(lg[:, j, :], ksb[off:off + hd, c, ts(kc, P)],
                               qsb[off:off + hd, c, ts(qi, 512)], True, True)
                        pt = ptpool.tile([P, 2, 512], cdt, tag="pt",
                                         name=f"pt{h}_{qi}_{kcp}")
                        nc.scalar.activation(pt[:], lg[:], Exp, scale=scale)
                        pts[kcp] = pt
                        drain((w, kcp, 1))
                        pull((w, kcp, 1))
                        if kcp > 0:
                            pj = pts[kcp - 1]
                            for j in range(2):
                                kc = 2 * (kcp - 1) + j
                                for qc in range(4):
                                    mm(pv[:, qc, 0:hd1],
                                       pj[:, j, ts(qc, P)],
                                       vsb[:, kc, h, :],
                                       kcp == 1 and j == 0 and qc == 0,
                                       False)
                    drain((w, NKP, 0))
                    # normalize by the ones-column sum into the pair tile;
                    # after the odd head, xbar-transpose the full 128-wide
                    # pair tile (transpose needs 128x128 xbar tiles).  The
                    # very last window pipelines per q-chunk straight into
                    # its own o-projection to shorten the drain tail.
                    last = (c == NC_ - 1 and qi == NQ - 1 and par == 1)
                    linv = linpool.tile([P, NQ], f32, tag="lin",
                                        name=f"linv{h}_{qi}")
                    if par == 0:
                        vals_pair[(c, qi)] = valspool.tile(
                            [P, NQ, 2 * hd], cdt, tag="vals",
                            name=f"vals{c}_{qi}")
                    vals = vals_pair[(c, qi)]
                    for j in range(2):
                        kc = 2 * (NKP - 1) + j
                        for qc in range(NQ):
                            mm(pv[:, qc, 0:hd1],
                               pts[NKP - 1][:, j, ts(qc, P)],
                               vsb[:, kc, h, :],
                               False, j == 1 and qc == NQ - 1)
                    for qc in range(NQ):
                        nc.vector.reciprocal(linv[:, qc:qc + 1],
                                             pv[:, qc, hd:hd1])
                        nc.vector.tensor_scalar_mul(vals[:, qc, off:off + hd],
                                                    pv[:, qc, 0:hd],
                                                    linv[:, qc:qc + 1])
                        if par == 1:
                            nc.sync.dma_start_transpose(
                                valsT[:, c, qi * 512 + qc * P:
                                      qi * 512 + (qc + 1) * P],
                                vals[:, qc, :])
                        if last and qc > 0:
                            o_stage2(qi, qc - 1, 0)()
                            o_stage2(qi, qc - 1, 1)()
                    if last:
                        o_stage2(qi, NQ - 1, 0)()
                        o_stage2(qi, NQ - 1, 1)()
            drain((NC_ * WPC, 0, 0))

    nc.compile()
    return nc


def _get_nc(S, D, H=16, SQ=None, use_bf16=True):
    key = (S, D, H, use_bf16)
    if key not in _NC_CACHE:
        _NC_CACHE[key] = _build_nc(S, D, H // 2, use_bf16)
    return _NC_CACHE[key]


def _host_prep(x, qkv_w, qkv_b, o_w, H, use_bf16=True):
    """Per-core weight slices: head-half hh gets heads 8hh..8hh+7."""
    import ml_dtypes
    wdt = ml_dtypes.bfloat16 if use_bf16 else np.float32
    D = o_w.shape[0]
    hd = D // H
    HC = H // 2
    DH = HC * hd
    qkv3 = qkv_w.reshape(H, 3, hd, D)
    b3 = qkv_b.reshape(H, 3, hd)
    shared = []
    for hh in range(2):
        hs = slice(hh * HC, (hh + 1) * HC)
        ds = slice(hh * DH, (hh + 1) * DH)
        shared.append(dict(
            wqT=np.ascontiguousarray(qkv3[hs, 0].reshape(DH, D).T.astype(wdt)),
            wkT=np.ascontiguousarray(qkv3[hs, 1].reshape(DH, D).T.astype(wdt)),
            wvT=np.ascontiguousarray(qkv3[hs, 2].reshape(DH, D).T.astype(wdt)),
            owT=np.ascontiguousarray(o_w.T[ds, :].astype(wdt)),
            bq=np.ascontiguousarray(b3[hs, 0].reshape(DH)),
            bk=np.ascontiguousarray(b3[hs, 1].reshape(DH)),
            bv=np.ascontiguousarray(b3[hs, 2].reshape(DH)),
        ))
    xTs = [np.ascontiguousarray(x[b].T.astype(wdt)) for b in range(x.shape[0])]
    return shared, xTs


def kernel(x, qkv_w, qkv_b, o_w, o_b, _trace=False):
    from concourse.bass_utils import run_bass_kernel_spmd

    x = np.asarray(x, dtype=np.float32)
    qkv_w = np.asarray(qkv_w, dtype=np.float32)
    qkv_b = np.asarray(qkv_b, dtype=np.float32)
    o_w = np.asarray(o_w, dtype=np.float32)
    o_b = np.asarray(o_b, dtype=np.float32)

    B, S, D = x.shape
    H = 16
    n_cores = 8

    nc = _get_nc(S, D, H)
    shared, xTs = _host_prep(x, qkv_w, qkv_b, o_w, H)

    in_maps = []
    for c in range(n_cores):
        b, hh = divmod(c, 2)
        m = dict(shared[hh])
        m["xT"] = xTs[b]
        in_maps.append(m)

    res = run_bass_kernel_spmd(nc, in_maps, list(range(n_cores)),
                               trace=_trace)

    out = np.empty((B, S, D), dtype=np.float32)
    for b in range(B):
        np.add(res.results[2 * b]["out"], res.results[2 * b + 1]["out"],
               out=out[b])
        out[b] += o_b
    if _trace:
        return out, res
    return out
